# Initial kernel scaffold
#
"""Trainium2 Bass kernel for nn_Convs4x44 (dense_cnn, memory-bound).

Pipeline per sample (64 input floats -> 4 output floats):
  conv1 2x2/s2 on 8x8 -> relu(x-0.2) -> conv2 2x2/s2 on 4x4 -> relu(x-2)
  -> 4->3 linear + relu -> 3->2 linear -> softmax(2)

Strategy: pure data parallel over 8 cores. Sample-major SBUF layout
[128 partitions, C samples/partition * 64 feats] so both elementwise engines
run with all 128 lanes busy and the HBM loads are big contiguous blocks.
Convs/MLP are fused scalar_tensor_tensor accumulation chains
(out = (x_tap * k) + acc) on DVE, with the leading product of each chain and
all ReLU/sigmoid on ACT; softmax(2) is computed exactly as
sigmoid(+/-(raw0-raw1)). GPSIMD is deliberately idle: its elementwise ops
measured 3-20x slower and its SBUF port contention slowed concurrent DVE ops
~3x.
"""

import numpy as np

import concourse.bass as bass
import concourse.tile as tile
from concourse import mybir
from concourse.bass_utils import run_bass_kernel_spmd


def _split_multiwaits(nc):
    """This container's walrus build supports only ONE sync-wait command per
    instruction ("Too many sync wait commands" otherwise), while Tile freely
    emits multi-wait instructions. Split every instruction with N>1 waits
    into (N-1) same-engine NoOps carrying one wait each, inserted before it
    in the basic block; per-engine execution order is block order filtered
    by engine, so semantics are unchanged."""
    for func in nc.m.functions:
        for blk in func.blocks:
            insts = blk.instructions
            out = []
            changed = False
            for ins in insts:
                si = ins.sync_info
                if si is not None and len(si.on_wait) > 1:
                    waits = list(si.on_wait)
                    for k, w in enumerate(waits[:-1]):
                        nop = mybir.InstNoOp(
                            name=f"{ins.name}-wsplit-{k}", ins=[], outs=[])
                        nop.engine = ins.engine
                        nop.sync_info = mybir.SyncInfo(on_wait=[w], on_update=[])
                        out.append(nop)
                    ins.sync_info = mybir.SyncInfo(
                        on_wait=[waits[-1]], on_update=list(si.on_update))
                    changed = True
                out.append(ins)
            if changed:
                insts[:] = out


N_CORES = 8
B = 1048576
BC = B // N_CORES          # samples per core
P = 128                    # SBUF partitions
# per-tile samples-per-partition: bigger tiles amortize per-op overhead;
# 5x192 + 64 covers BC = 131072 = 128 * 1024
TILE_CS = [192, 192, 192, 192, 192, 64]
assert sum(TILE_CS) * P == BC

F32 = mybir.dt.float32
ALU = mybir.AluOpType
AF = mybir.ActivationFunctionType

# columns in the broadcast-constant tile
K1 = 0            # conv1_w taps  [k00,k01,k10,k11]
K2 = 4            # conv2_w taps
W1C = 8           # W1[j,i] -> 8 + 4j + i
B1C = 20          # b1[j]
W2C = 23          # W2[j,i] -> 23 + 3j + i
B2C = 29          # b2[j]
SH1 = 31          # -0.2 (conv1 relu shift)
SH2 = 32          # -2.0 (f relu shift)
NW = 33


def _build():
    nc = bass.Bass("TRN2", target_bir_lowering=False, debug=False,
                   num_devices=N_CORES)
    x = nc.dram_tensor("x", [BC, 64], F32, kind="ExternalInput")
    wconst = nc.dram_tensor("wconst", [P, NW], F32, kind="ExternalInput")
    out = nc.dram_tensor("out", [BC, 4], F32, kind="ExternalOutput")

    with tile.TileContext(nc) as tc:
        with (
            tc.tile_pool(name="consts", bufs=1) as cpool,
            tc.tile_pool(name="x", bufs=2) as xpool,
            tc.tile_pool(name="mid", bufs=2) as mpool,
            tc.tile_pool(name="small", bufs=2) as spool,
            tc.tile_pool(name="out", bufs=2) as opool,
        ):
            ws = cpool.tile([P, NW], F32)
            nc.sync.dma_start(ws[:], wconst.ap()[:])

            def sc(col):
                return ws[:, col:col + 1]

            CMAX = max(TILE_CS)
            s0 = 0
            for C in TILE_CS:
                ns = P * C
                x_view = x.ap()[s0:s0 + ns, :].rearrange(
                    "(p c) f -> p (c f)", p=P, c=C)
                out_view = out.ap()[s0:s0 + ns, :].rearrange(
                    "(p c) four -> p (c four)", p=P, c=C)
                s0 += ns

                xt = xpool.tile([P, CMAX * 64], F32, tag="xt")
                nc.sync.dma_start(xt[:, :C * 64], x_view)

                # conv1: t1[c,oh,ow] = sum_taps k[ti,tj] * x[c,2oh+ti,2ow+tj]
                xv = xt[:, :C * 64].rearrange(
                    "p (c oh ti ow tj) -> p c oh ti ow tj", oh=4, ti=2, ow=4,
                    tj=2)
                t1 = mpool.tile([P, CMAX * 16], F32, tag="t1")
                t1v = t1[:, :C * 16].rearrange("p (c oh ow) -> p c oh ow",
                                               oh=4, ow=4)
                nc.scalar.activation(t1v, xv[:, :, :, 0, :, 0], AF.Copy,
                                     bias=0.0, scale=sc(K1 + 0))
                nc.vector.scalar_tensor_tensor(t1v, xv[:, :, :, 0, :, 1],
                                               sc(K1 + 1), t1v, ALU.mult, ALU.add)
                nc.vector.scalar_tensor_tensor(t1v, xv[:, :, :, 1, :, 0],
                                               sc(K1 + 2), t1v, ALU.mult, ALU.add)
                nc.vector.scalar_tensor_tensor(t1v, xv[:, :, :, 1, :, 1],
                                               sc(K1 + 3), t1v, ALU.mult, ALU.add)

                # relu(x1 - 0.2)
                x1r = mpool.tile([P, CMAX * 16], F32, tag="x1r")
                nc.scalar.activation(x1r[:, :C * 16], t1[:, :C * 16], AF.Relu,
                                     bias=sc(SH1), scale=1.0)

                # conv2 on the 4x4 maps
                x1v = x1r[:, :C * 16].rearrange(
                    "p (c oh ti ow tj) -> p c oh ti ow tj", oh=2, ti=2, ow=2,
                    tj=2)
                t2 = spool.tile([P, CMAX * 4], F32, tag="t2")
                t2v = t2[:, :C * 4].rearrange("p (c oh ow) -> p c oh ow",
                                              oh=2, ow=2)
                nc.scalar.activation(t2v, x1v[:, :, :, 0, :, 0], AF.Copy,
                                     bias=0.0, scale=sc(K2 + 0))
                nc.vector.scalar_tensor_tensor(t2v, x1v[:, :, :, 0, :, 1],
                                               sc(K2 + 1), t2v, ALU.mult, ALU.add)
                nc.vector.scalar_tensor_tensor(t2v, x1v[:, :, :, 1, :, 0],
                                               sc(K2 + 2), t2v, ALU.mult, ALU.add)
                nc.vector.scalar_tensor_tensor(t2v, x1v[:, :, :, 1, :, 1],
                                               sc(K2 + 3), t2v, ALU.mult, ALU.add)

                # f = relu(x2 - 2)
                f = spool.tile([P, CMAX * 4], F32, tag="f")
                nc.scalar.activation(f[:, :C * 4], t2[:, :C * 4], AF.Relu,
                                     bias=sc(SH2), scale=1.0)
                fv = f[:, :C * 4].rearrange("p (c i) -> p c i", i=4)

                # h_j = relu(sum_i W1[j,i] f_i + b1_j), stored j-major
                h = spool.tile([P, CMAX * 3], F32, tag="h")
                for j in range(3):
                    hj = h[:, j * C:(j + 1) * C]
                    nc.scalar.activation(hj, fv[:, :, 0], AF.Identity,
                                         bias=sc(B1C + j), scale=sc(W1C + 4 * j))
                    for i in range(1, 4):
                        nc.vector.scalar_tensor_tensor(
                            hj, fv[:, :, i], sc(W1C + 4 * j + i), hj,
                            ALU.mult, ALU.add)
                hr = spool.tile([P, CMAX * 3], F32, tag="hr")
                nc.scalar.activation(hr[:, :C * 3], h[:, :C * 3], AF.Relu,
                                     bias=0.0, scale=1.0)
                hrv = hr[:, :C * 3].rearrange("p (j c) -> p j c", j=3)

                # out tile layout per sample: [cls0, cls1, raw0, raw1]
                ot = opool.tile([P, CMAX * 4], F32, tag="ot")
                ov = ot[:, :C * 4].rearrange("p (c four) -> p c four", four=4)
                for j in range(2):
                    rj = ov[:, :, 2 + j]
                    nc.scalar.activation(rj, hrv[:, 0, :], AF.Identity,
                                         bias=sc(B2C + j), scale=sc(W2C + 3 * j))
                    for i in range(1, 3):
                        nc.vector.scalar_tensor_tensor(
                            rj, hrv[:, i, :], sc(W2C + 3 * j + i), rj,
                            ALU.mult, ALU.add)

                # softmax over 2 classes: cls0 = sigmoid(raw0-raw1)
                d = spool.tile([P, CMAX], F32, tag="d")
                nc.vector.tensor_sub(d[:, :C], ov[:, :, 2], ov[:, :, 3])
                nc.scalar.activation(ov[:, :, 0], d[:, :C], AF.Sigmoid,
                                     bias=0.0, scale=1.0)
                nc.scalar.activation(ov[:, :, 1], d[:, :C], AF.Sigmoid,
                                     bias=0.0, scale=-1.0)

                nc.sync.dma_start(out_view, ot[:, :C * 4])

    _split_multiwaits(nc)
    return nc


_NC = None


def _get_nc():
    global _NC
    if _NC is None:
        _NC = _build()
    return _NC


def kernel(x, conv1_w, conv2_w, W1, b1, W2, b2):
    x = np.ascontiguousarray(np.asarray(x, dtype=np.float32)).reshape(B, 64)
    row = np.concatenate([
        np.asarray(conv1_w, dtype=np.float32).reshape(4),
        np.asarray(conv2_w, dtype=np.float32).reshape(4),
        np.asarray(W1, dtype=np.float32).reshape(12),
        np.asarray(b1, dtype=np.float32).reshape(3),
        np.asarray(W2, dtype=np.float32).reshape(6),
        np.asarray(b2, dtype=np.float32).reshape(2),
        np.array([-0.2, -2.0], dtype=np.float32),
    ])
    wconst = np.ascontiguousarray(np.tile(row[None, :], (P, 1)))

    nc = _get_nc()
    in_maps = [
        {"x": np.ascontiguousarray(x[i * BC:(i + 1) * BC]), "wconst": wconst}
        for i in range(N_CORES)
    ]
    res = run_bass_kernel_spmd(nc, in_maps, core_ids=list(range(N_CORES)))
    out = np.concatenate([res.results[i]["out"] for i in range(N_CORES)], axis=0)
    classification = np.ascontiguousarray(out[:, 0:2])
    raw = np.ascontiguousarray(out[:, 2:4])
    return classification, raw



# revision 4
# speedup vs baseline: 72.1691x; 72.1691x over previous
"""Trainium2 Bass kernel for nn_Convs4x44 (dense_cnn, memory-bound).

Pipeline per sample (64 input floats -> 4 output floats):
  conv1 2x2/s2 on 8x8 -> relu(x-0.2) -> conv2 2x2/s2 on 4x4 -> relu(x-2)
  -> 4->3 linear + relu -> 3->2 linear -> softmax(2)

Strategy: pure data parallel over 8 cores. Sample-major SBUF layout
[128 partitions, C samples/partition * 64 feats] so both elementwise engines
run with all 128 lanes busy and the HBM loads are big contiguous blocks.
Convs/MLP are fused scalar_tensor_tensor accumulation chains
(out = (x_tap * k) + acc) on DVE, with the leading product of each chain and
all ReLU/sigmoid on ACT; softmax(2) is computed exactly as
sigmoid(+/-(raw0-raw1)). GPSIMD is deliberately idle: its elementwise ops
measured 3-20x slower and its SBUF port contention slowed concurrent DVE ops
~3x.
"""

import numpy as np

import concourse.bass as bass
import concourse.tile as tile
from concourse import mybir
from concourse.bass_utils import run_bass_kernel_spmd


def _split_multiwaits(nc):
    """This container's walrus build supports only ONE sync-wait command per
    instruction ("Too many sync wait commands" otherwise), while Tile freely
    emits multi-wait instructions. Split every instruction with N>1 waits
    into (N-1) same-engine NoOps carrying one wait each, inserted before it
    in the basic block; per-engine execution order is block order filtered
    by engine, so semantics are unchanged."""
    for func in nc.m.functions:
        for blk in func.blocks:
            insts = blk.instructions
            out = []
            changed = False
            for ins in insts:
                si = ins.sync_info
                if si is not None and len(si.on_wait) > 1:
                    waits = list(si.on_wait)
                    for k, w in enumerate(waits[:-1]):
                        nop = mybir.InstNoOp(
                            name=f"{ins.name}-wsplit-{k}", ins=[], outs=[])
                        nop.engine = ins.engine
                        nop.sync_info = mybir.SyncInfo(on_wait=[w], on_update=[])
                        out.append(nop)
                    ins.sync_info = mybir.SyncInfo(
                        on_wait=[waits[-1]], on_update=list(si.on_update))
                    changed = True
                out.append(ins)
            if changed:
                insts[:] = out


N_CORES = 8
B = 1048576
BC = B // N_CORES          # samples per core
P = 128                    # SBUF partitions
# per-tile samples-per-partition: bigger tiles amortize per-op overhead;
# 5x192 + 64 covers BC = 131072 = 128 * 1024
TILE_CS = [192, 192, 192, 192, 192, 64]
assert sum(TILE_CS) * P == BC

F32 = mybir.dt.float32
ALU = mybir.AluOpType
AF = mybir.ActivationFunctionType

# columns in the broadcast-constant tile
K1 = 0            # conv1_w taps  [k00,k01,k10,k11]
K2 = 4            # conv2_w taps
W1C = 8           # W1[j,i] -> 8 + 4j + i
B1C = 20          # b1[j]
W2C = 23          # W2[j,i] -> 23 + 3j + i
B2C = 29          # b2[j]
SH1 = 31          # -0.2 (conv1 relu shift)
SH2 = 32          # -2.0 (f relu shift)
NW = 33


def _build():
    nc = bass.Bass("TRN2", target_bir_lowering=False, debug=False,
                   num_devices=N_CORES)
    x = nc.dram_tensor("x", [BC, 64], F32, kind="ExternalInput")
    wconst = nc.dram_tensor("wconst", [P, NW], F32, kind="ExternalInput")
    out = nc.dram_tensor("out", [BC, 4], F32, kind="ExternalOutput")

    with tile.TileContext(nc) as tc:
        with (
            tc.tile_pool(name="consts", bufs=1) as cpool,
            tc.tile_pool(name="x", bufs=2) as xpool,
            tc.tile_pool(name="mid", bufs=2) as mpool,
            tc.tile_pool(name="small", bufs=2) as spool,
            tc.tile_pool(name="out", bufs=2) as opool,
        ):
            ws = cpool.tile([P, NW], F32)
            nc.sync.dma_start(ws[:], wconst.ap()[:])

            def sc(col):
                return ws[:, col:col + 1]

            CMAX = max(TILE_CS)
            s0 = 0
            for C in TILE_CS:
                ns = P * C
                x_view = x.ap()[s0:s0 + ns, :].rearrange(
                    "(p c) f -> p (c f)", p=P, c=C)
                out_view = out.ap()[s0:s0 + ns, :].rearrange(
                    "(p c) four -> p (c four)", p=P, c=C)
                s0 += ns

                xt = xpool.tile([P, CMAX * 64], F32, tag="xt")
                nc.sync.dma_start(xt[:, :C * 64], x_view)

                # conv1: t1[c,oh,ow] = sum_taps k[ti,tj] * x[c,2oh+ti,2ow+tj]
                xv = xt[:, :C * 64].rearrange(
                    "p (c oh ti ow tj) -> p c oh ti ow tj", oh=4, ti=2, ow=4,
                    tj=2)
                t1 = mpool.tile([P, CMAX * 16], F32, tag="t1")
                t1v = t1[:, :C * 16].rearrange("p (c oh ow) -> p c oh ow",
                                               oh=4, ow=4)
                nc.scalar.activation(t1v, xv[:, :, :, 0, :, 0], AF.Copy,
                                     bias=0.0, scale=sc(K1 + 0))
                nc.vector.scalar_tensor_tensor(t1v, xv[:, :, :, 0, :, 1],
                                               sc(K1 + 1), t1v, ALU.mult, ALU.add)
                nc.vector.scalar_tensor_tensor(t1v, xv[:, :, :, 1, :, 0],
                                               sc(K1 + 2), t1v, ALU.mult, ALU.add)
                nc.vector.scalar_tensor_tensor(t1v, xv[:, :, :, 1, :, 1],
                                               sc(K1 + 3), t1v, ALU.mult, ALU.add)

                # relu(x1 - 0.2)
                x1r = mpool.tile([P, CMAX * 16], F32, tag="x1r")
                nc.scalar.activation(x1r[:, :C * 16], t1[:, :C * 16], AF.Relu,
                                     bias=sc(SH1), scale=1.0)

                # conv2 on the 4x4 maps
                x1v = x1r[:, :C * 16].rearrange(
                    "p (c oh ti ow tj) -> p c oh ti ow tj", oh=2, ti=2, ow=2,
                    tj=2)
                t2 = spool.tile([P, CMAX * 4], F32, tag="t2")
                t2v = t2[:, :C * 4].rearrange("p (c oh ow) -> p c oh ow",
                                              oh=2, ow=2)
                nc.scalar.activation(t2v, x1v[:, :, :, 0, :, 0], AF.Copy,
                                     bias=0.0, scale=sc(K2 + 0))
                nc.vector.scalar_tensor_tensor(t2v, x1v[:, :, :, 0, :, 1],
                                               sc(K2 + 1), t2v, ALU.mult, ALU.add)
                nc.vector.scalar_tensor_tensor(t2v, x1v[:, :, :, 1, :, 0],
                                               sc(K2 + 2), t2v, ALU.mult, ALU.add)
                nc.vector.scalar_tensor_tensor(t2v, x1v[:, :, :, 1, :, 1],
                                               sc(K2 + 3), t2v, ALU.mult, ALU.add)

                # f = relu(x2 - 2)
                f = spool.tile([P, CMAX * 4], F32, tag="f")
                nc.scalar.activation(f[:, :C * 4], t2[:, :C * 4], AF.Relu,
                                     bias=sc(SH2), scale=1.0)
                fv = f[:, :C * 4].rearrange("p (c i) -> p c i", i=4)

                # h_j = relu(sum_i W1[j,i] f_i + b1_j), stored j-major
                h = spool.tile([P, CMAX * 3], F32, tag="h")
                for j in range(3):
                    hj = h[:, j * C:(j + 1) * C]
                    nc.scalar.activation(hj, fv[:, :, 0], AF.Identity,
                                         bias=sc(B1C + j), scale=sc(W1C + 4 * j))
                    for i in range(1, 4):
                        nc.vector.scalar_tensor_tensor(
                            hj, fv[:, :, i], sc(W1C + 4 * j + i), hj,
                            ALU.mult, ALU.add)
                hr = spool.tile([P, CMAX * 3], F32, tag="hr")
                nc.scalar.activation(hr[:, :C * 3], h[:, :C * 3], AF.Relu,
                                     bias=0.0, scale=1.0)
                hrv = hr[:, :C * 3].rearrange("p (j c) -> p j c", j=3)

                # out tile layout per sample: [cls0, cls1, raw0, raw1]
                ot = opool.tile([P, CMAX * 4], F32, tag="ot")
                ov = ot[:, :C * 4].rearrange("p (c four) -> p c four", four=4)
                for j in range(2):
                    rj = ov[:, :, 2 + j]
                    nc.scalar.activation(rj, hrv[:, 0, :], AF.Identity,
                                         bias=sc(B2C + j), scale=sc(W2C + 3 * j))
                    for i in range(1, 3):
                        nc.vector.scalar_tensor_tensor(
                            rj, hrv[:, i, :], sc(W2C + 3 * j + i), rj,
                            ALU.mult, ALU.add)

                # softmax over 2 classes: cls0 = sigmoid(raw0-raw1)
                d = spool.tile([P, CMAX], F32, tag="d")
                nc.vector.tensor_sub(d[:, :C], ov[:, :, 2], ov[:, :, 3])
                nc.scalar.activation(ov[:, :, 0], d[:, :C], AF.Sigmoid,
                                     bias=0.0, scale=1.0)
                nc.scalar.activation(ov[:, :, 1], d[:, :C], AF.Sigmoid,
                                     bias=0.0, scale=-1.0)

                nc.sync.dma_start(out_view, ot[:, :C * 4])

    _split_multiwaits(nc)
    return nc


_NC = None
LAST_RESULT = None  # BassKernelResults of the most recent kernel() call


def _get_nc():
    global _NC
    if _NC is None:
        _NC = _build()
    return _NC


def prep_in_maps(x, conv1_w, conv2_w, W1, b1, W2, b2):
    x = np.ascontiguousarray(np.asarray(x, dtype=np.float32)).reshape(B, 64)
    row = np.concatenate([
        np.asarray(conv1_w, dtype=np.float32).reshape(4),
        np.asarray(conv2_w, dtype=np.float32).reshape(4),
        np.asarray(W1, dtype=np.float32).reshape(12),
        np.asarray(b1, dtype=np.float32).reshape(3),
        np.asarray(W2, dtype=np.float32).reshape(6),
        np.asarray(b2, dtype=np.float32).reshape(2),
        np.array([-0.2, -2.0], dtype=np.float32),
    ])
    wconst = np.ascontiguousarray(np.tile(row[None, :], (P, 1)))
    return [
        {"x": np.ascontiguousarray(x[i * BC:(i + 1) * BC]), "wconst": wconst}
        for i in range(N_CORES)
    ]


def kernel(x, conv1_w, conv2_w, W1, b1, W2, b2):
    in_maps = prep_in_maps(x, conv1_w, conv2_w, W1, b1, W2, b2)
    nc = _get_nc()
    res = run_bass_kernel_spmd(nc, in_maps, core_ids=list(range(N_CORES)))
    global LAST_RESULT
    LAST_RESULT = res
    out = np.concatenate([res.results[i]["out"] for i in range(N_CORES)], axis=0)
    classification = np.ascontiguousarray(out[:, 0:2])
    raw = np.ascontiguousarray(out[:, 2:4])
    return classification, raw

